# revision 38
# baseline (speedup 1.0000x reference)
"""Trainium2 Bass kernel for nn_DiffIoU: differentiable polygon/mask IoU.

Strategy (v2):
- Data-parallel over batch: 8 NeuronCores x 64 examples.
- "Crossing form": the reference's floor-dedup keeps exactly one sample per
  integer cell of the dedup axis u. Because the crossing predicate reduces to
  k >= t with t = (cell + desc - u0)*(1/vu) evaluated at f32 precision,
  the first sample index is exactly k = ceil(t) -- no probe iterations.
  Borderline ulp mismatches vs the reference's u-space dedup are absorbed by
  the final fu in [0,1) validity check.
- Per-point math runs on [128 pass x 800 (example,cell)] tiles, 8 streams per
  set, with per-(pass,example) scalars applied via stride-0 broadcast views.
- Masks are fp16 tables, 8 combos per stream (2 masks x 4 shifted copies
  {0,1,100,101} for the bilinear taps); one shared index stream per
  (example, axis) feeds all 8 via GPSIMD indirect_copy (1024-idx chunks
  reading 1024-word windows; stream order i = cell*128 + pass).
- Index wrap into the gather's 16-partition layout is done on the PE via a
  double transpose (no DMA descriptors), with a p1-major staging layout.
- Hop: 64 DMAs/set scatter gather rows into per-mask HT tiles [100 x 4096];
  weights (fp16, f32 chain with one final cast) are PE-transposed into one
  PSUM tile, copied once to SBUF; per mask f32 products + a segmented DVE
  reduce (mask 0) / ACT accum_out reductions (mask 1) finish the sums, and
  the final ones-matmul is folded per-set onto the idle PE.
"""
import os
import re as _re
import numpy as np
import ml_dtypes

import concourse.bass as bass
import concourse.mybir as mybir
from concourse import tile


def _vc_vals(vc):
    m = _re.search(r"VectorClock\(\[(.*)\]\)", repr(vc))
    return [int(x) for x in m.group(1).split(",")]


def _patched_drain_and_barrier(self, tick_clock, wait_clock):
    # This walrus build allows very few sync-wait slots per instruction;
    # Tile's stock tail drain stacks one wait per live semaphore on a single
    # CTRL instruction and overflows it. Emit single-wait instructions.
    vals = _vc_vals(tick_clock.global_clock)
    for proc, sem in sorted(wait_clock.sems.allocated().items()):
        ticks = vals[proc] if proc < len(vals) else 0
        if ticks > 0:
            mult = 16 if sem.name.startswith("DMA") else 1
            self.nc.sync.wait_ge(sem, ticks * mult)
    self.nc.sync.drain()
    self.nc.all_engine_barrier()
    assert self.sems is not None
    popped = self.nc._tile_sem_poison_stack.pop()
    assert popped is self._sem_poison
    sems = list(self.sems.allocated().values())
    for i in range(0, len(sems), 8):
        self.nc.clear_and_free_semaphores(sems[i:i + 8])
    self.nc.all_engine_barrier()


tile.TileContext._drain_and_barrier = _patched_drain_and_barrier


def _split_excess_waits(nc, cap=1):
    # Walrus in this container allows only ~3 sync-wait slots per instruction.
    # Move excess waits onto injected same-engine NoOps placed just before.
    for fn in nc.m.functions:
        for bb in fn.blocks:
            lst = bb.instructions
            i = 0
            while i < len(lst):
                ins = lst[i]
                si = ins.sync_info
                if si and si.on_wait and len(si.on_wait) > cap:
                    waits = list(si.on_wait)
                    extra, keep = waits[:-cap], waits[-cap:]
                    ins.sync_info = mybir.SyncInfo(
                        on_wait=keep, on_update=list(si.on_update or []))
                    nops = []
                    for j in range(0, len(extra), cap):
                        nop = mybir.InstDrain(
                            name=f"{ins.name}_wsplit{j}", ins=[], outs=[])
                        nop.engine = ins.engine
                        nop.sync_info = mybir.SyncInfo(
                            on_wait=extra[j:j + cap], on_update=[])
                        nops.append(nop)
                    for k, nop in enumerate(nops):
                        lst.insert(i + k, nop)
                    i += len(nops)
                i += 1


F32 = mybir.dt.float32
BF16 = mybir.dt.bfloat16
U16 = mybir.dt.uint16
FP8 = mybir.dt.float8e4
F16 = mybir.dt.float16
ALU = mybir.AluOpType
ACTF = mybir.ActivationFunctionType
AXF = mybir.AxisListType.X

DIM = 100
NV = 64
NPASS = 128            # 64 edges x {fwd, bwd}
NCELL = 100
NIDX = NPASS * NCELL   # 12800 indices per stream
CHUNK = 1024
NCHUNK = (NIDX + CHUNK - 1) // CHUNK   # 13
WINDOW = 1024
TABLE = 10752          # shifted per-partition table length (packed words)
TAB_SRC = 10880        # host-side packed array length
N_CORES = 8
SHIFTS = (0, 1, 100, 101)
MAGIC = float(2 ** 23)
MAGIC2 = float(3 * 2 ** 22)   # two-sided round magic, exact for |t| <= 2^22

_cache = {}


def build_module(b_core: int, body_reps: int = 1, debug_dump: bool = False):
    DUP = set(os.environ.get("KDUP", "").split(","))
    nstream = 2 * b_core          # stream = ax*b_core + b (ax-major)
    nset = nstream // 8
    nc = bass.Bass()

    def din(name, shape, dt=F32):
        return nc.declare_dram_parameter(name, list(shape), dt, isOutput=False)

    X0 = din("X0", (NPASS, b_core)); Y0 = din("Y0", (NPASS, b_core))
    VX = din("VX", (NPASS, b_core)); VY = din("VY", (NPASS, b_core))
    X1 = din("X1", (NPASS, b_core)); Y1 = din("Y1", (NPASS, b_core))
    PAd = [din(f"PA{i}", (b_core, NV)) for i in range(4)]
    GAd = [din(f"GA{i}", (b_core, NV)) for i in range(4)]
    TBL = din("TBL", (b_core, 2, 2, TAB_SRC), F16)  # fp16 tables
    CELLS8 = din("CELLS8", (NPASS, 800))
    EB8 = din("EB8", (NPASS, 800))
    FLIP = din("FLIP", (NPASS, 1))
    ONES = din("ONES", (NPASS, 1))
    IDN = din("IDN", (NPASS, NPASS))
    IDNB = din("IDNB", (NPASS, NPASS), F16)
    OUT = nc.declare_dram_parameter("IOU", [1, b_core], F32, isOutput=True)
    dbg = {}
    if debug_dump:
        dbg["W0"] = nc.declare_dram_parameter("DBG_W0", [NPASS, 800], F16, isOutput=True)
        dbg["EU"] = nc.declare_dram_parameter("DBG_EU", [NPASS, 800], U16, isOutput=True)
        dbg["HT"] = nc.declare_dram_parameter("DBG_HT", [NCELL, 8192], F16, isOutput=True)
        dbg["CO"] = nc.declare_dram_parameter("DBG_CO", [NCELL, 1024], F32, isOutput=True)
        dbg["WT"] = nc.declare_dram_parameter("DBG_WT", [NCELL, 4096], F16, isOutput=True)
        dbg["P0"] = nc.declare_dram_parameter("DBG_P0", [NCELL, 2048], F32, isOutput=True)
        dbg["FU"] = nc.declare_dram_parameter("DBG_FU", [NPASS, 800], F32, isOutput=True)
        dbg["WW"] = nc.declare_dram_parameter("DBG_WW", [NPASS, 800], F32, isOutput=True)
        dbg["VA"] = nc.declare_dram_parameter("DBG_VA", [NPASS, 800], F32, isOutput=True)
        dbg["KF"] = nc.declare_dram_parameter("DBG_KF", [NPASS, 800], F32, isOutput=True)

    with tile.TileContext(nc) as tc:
        with tc.tile_pool(name="sb", bufs=1) as P, \
             tc.tile_pool(name="sb2", bufs=2) as P2, \
             tc.tile_pool(name="ps", bufs=1, space="PSUM") as PS:
          for _rep in range(body_reps):
            f = float

            def TS(out, in0, s1, s2, op0, op1=None, eng=None):
                e = eng or nc.vector
                if op1 is None:
                    e.tensor_scalar(out, in0, s1, s2, op0)
                else:
                    e.tensor_scalar(out, in0, s1, s2, op0, op1)

            def TT(out, a, b_, op, eng=None):
                (eng or nc.vector).tensor_tensor(out, a, b_, op)

            # ---------- small inputs ----------
            def load(dram, shape, dt=F32):
                t = P.tile(list(shape), dt, tag=f"in_{dram.name}",
                           name=f"in_{dram.name}")
                nc.sync.dma_start(t[:], dram[:])
                return t

            x0 = load(X0, (NPASS, b_core)); y0 = load(Y0, (NPASS, b_core))
            vx = load(VX, (NPASS, b_core)); vy = load(VY, (NPASS, b_core))
            x1 = load(X1, (NPASS, b_core)); y1 = load(Y1, (NPASS, b_core))
            cells8 = load(CELLS8, (NPASS, 800))
            eb8 = load(EB8, (NPASS, 800))
            flip = load(FLIP, (NPASS, 1))
            ones = load(ONES, (NPASS, 1))
            idn = load(IDN, (NPASS, NPASS))
            idnb = load(IDNB, (NPASS, NPASS), F16)
            pa = [load(d, (b_core, NV)) for d in PAd]
            ga = [load(d, (b_core, NV)) for d in GAd]

            # wait-absorber: soak up the input-DMA waits pairwise on DVE.
            absorb = P.tile([1, 2], F32, tag="absorb", name="absorb")
            for t_a, t_b in ((x0, y0), (x1, y1), (vx, vy), (cells8, eb8),
                             (flip, ones), (idn, pa[0]), (pa[1], pa[2]),
                             (pa[3], ga[0]), (ga[1], ga[2]),
                             (ga[3], idnb.bitcast(F32))):
                TT(absorb[:, 0:1], t_a[0:1, 0:1], t_b[0:1, 0:1], ALU.add)

            # ---------- per-edge prep on [128 x b] ----------
            def eb(tag):
                return P.tile([NPASS, b_core], F32, tag=tag, name=tag)

            ivx = eb("ivx"); nc.vector.reciprocal(ivx[:], vx[:])
            ivy = eb("ivy"); nc.vector.reciprocal(ivy[:], vy[:])
            xhi = eb("xhi"); TT(xhi[:], x0[:], x1[:], ALU.max)
            xlo = eb("xlo"); TT(xlo[:], x0[:], x1[:], ALU.min)
            yhi = eb("yhi"); TT(yhi[:], y0[:], y1[:], ALU.max)
            ylo = eb("ylo"); TT(ylo[:], y0[:], y1[:], ALU.min)

            def k_upper(tag, hi, lo, o, iv):
                a = eb(tag + "a"); bt = eb(tag + "b")
                TT(a[:], hi[:], o[:], ALU.subtract)
                TS(a[:], a[:], f(1e-3), None, ALU.add)
                TT(a[:], a[:], iv[:], ALU.mult)
                TT(bt[:], lo[:], o[:], ALU.subtract)
                TS(bt[:], bt[:], f(1e-3), None, ALU.subtract)
                TT(bt[:], bt[:], iv[:], ALU.mult)
                TT(a[:], a[:], bt[:], ALU.max)
                return a

            Kx = k_upper("kx", xhi, xlo, x0, ivx)
            Ky = k_upper("ky", yhi, ylo, y0, ivy)

            def k_bound(tag, o, iv):
                a = eb(tag + "a"); bt = eb(tag + "b")
                TS(a[:], o[:], f(-1.0), f(99.0), ALU.mult, ALU.add)
                TT(a[:], a[:], iv[:], ALU.mult)
                TS(bt[:], o[:], f(-1.0), None, ALU.mult)
                TT(bt[:], bt[:], iv[:], ALU.mult)
                TT(a[:], a[:], bt[:], ALU.max)
                return a

            Kux = k_bound("kux", x0, ivx)
            Kuy = k_bound("kuy", y0, ivy)
            khix = eb("khix"); TT(khix[:], Kx[:], Ky[:], ALU.min)
            TT(khix[:], khix[:], Kux[:], ALU.min)
            TS(khix[:], khix[:], f(200.0), None, ALU.min)
            khiy = eb("khiy"); TT(khiy[:], Kx[:], Ky[:], ALU.min)
            TT(khiy[:], khiy[:], Kuy[:], ALU.min)
            TS(khiy[:], khiy[:], f(200.0), None, ALU.min)

            def sign_half(tag, p1, p0):
                c = eb(tag)
                TT(c[:], p1[:], p0[:], ALU.is_gt)           # {0,1}
                TS(c[:], c[:], f(0.5), None, ALU.subtract)  # +-0.5
                nc.vector.tensor_scalar(c[:], c[:], flip[:], None, ALU.mult)
                return c

            cxs = sign_half("cxs", x1, x0)
            cys = sign_half("cys", y1, y0)
            c99 = P.tile([NPASS, 1], F32, tag="c99", name="c99")
            nc.vector.memset(c99[:], 99.0)
            descx = eb("descx"); TS(descx[:], vx[:], f(0.0), None, ALU.is_lt)
            descy = eb("descy"); TS(descy[:], vy[:], f(0.0), None, ALU.is_lt)
            dix = eb("dix"); TT(dix[:], descx[:], ivx[:], ALU.mult)
            diy = eb("diy"); TT(diy[:], descy[:], ivy[:], ALU.mult)
            qsx = eb("qsx"); TS(qsx[:], descx[:], f(-2.0), f(1.0), ALU.mult,
                               ALU.add)
            qsy = eb("qsy"); TS(qsy[:], descy[:], f(-2.0), f(1.0), ALU.mult,
                               ALU.add)

            # ---------- areas ----------
            def area(tag, t4):
                ymax = P.tile([b_core, 1], F32, tag=tag + "ym", name=tag + "ym")
                nc.vector.tensor_reduce(ymax[:], t4[1][:], AXF, ALU.max)
                yy = P.tile([b_core, NV], F32, tag=tag + "yy", name=tag + "yy")
                TT(yy[:], t4[3][:], t4[1][:], ALU.add)
                nc.vector.tensor_scalar(yy[:], yy[:], f(-0.5), ymax[:],
                                        ALU.mult, ALU.add)
                dxe = P.tile([b_core, NV], F32, tag=tag + "dx", name=tag + "dx")
                TT(dxe[:], t4[2][:], t4[0][:], ALU.subtract)
                TT(yy[:], yy[:], dxe[:], ALU.mult)
                s = P.tile([b_core, 1], F32, tag=tag + "s", name=tag + "s")
                nc.vector.tensor_reduce(s[:], yy[:], AXF, ALU.add)
                sa = P.tile([b_core, 1], F32, tag=tag + "sa", name=tag + "sa")
                nc.scalar.activation(sa[:], s[:], ACTF.Abs)
                return sa

            pred_a = area("pa", pa)
            gt_a = area("ga", ga)

            # ---------- per-set streams ----------
            collect = P.tile([NCELL, nset * 64], F32, tag="collect",
                             name="collect")
            nc.vector.memset(collect[:], 0.0)
            tsum = P.tile([1, nset * 64], F32, tag="tsum", name="tsum")
            gout = P.tile([NPASS, NIDX], F16, tag="gout", name="gout")
            table_t = P2.tile([NPASS, TABLE], F16, tag="tables", name="tables")
            idxw_bufs = [P2.tile([NPASS, NIDX // 16], U16, tag=f"idxw{i}",
                                 name=f"idxw{i}", bufs=1) for i in range(2)]
            scratch = P.tile([NPASS, 4], F32, tag="scratch", name="scratch")

            def big(tag, dt=F32):
                return P2.tile([NPASS, 800], dt, tag=tag, name=tag)

            def v3(t):
                return t[:].rearrange("p (a b) -> p a b", a=8)

            for s_i in range(nset):
                ax = (s_i * 8) // b_core
                b0 = (s_i * 8) % b_core
                idxw = idxw_bufs[s_i % 2]
                u0s, w0s = (x0, y0) if ax == 0 else (y0, x0)
                vus, vws = (vx, vy) if ax == 0 else (vy, vx)
                ivus = ivx if ax == 0 else ivy
                dis = dix if ax == 0 else diy
                khis = khix if ax == 0 else khiy
                cs = cxs if ax == 0 else cys
                descs = descx if ax == 0 else descy
                qs = qsx if ax == 0 else qsy

                def bc(tt):
                    return tt[:, b0:b0 + 8].broadcast_to((NPASS, 8, 100))

                # shifted combo tables: one batched DMA per combo q
                for _dup in range(2 if "tbl" in DUP else 1):
                    for q in range(8):
                        m_q, j_q = divmod(q, 4)
                        nc.scalar.dma_start(
                            table_t[q:128:16, :],
                            TBL[b0:b0 + 8, ax, m_q,
                                SHIFTS[j_q]:SHIFTS[j_q] + TABLE],
                        )

                # ---- per-point math on [128 x 800] ----
                TA = big("TA"); TB = big("TB"); TC = big("TC")
                TD = big("TD"); TE = big("TE"); TF = big("TF")
                TG = big("TG"); TH = big("TH"); TI = big("TI")
                u0c = TA; TT(v3(u0c), bc(u0s), v3(cells8), ALU.subtract)
                m0 = TB; TT(v3(m0), v3(u0c), bc(ivus), ALU.mult)
                t_ = TC; TT(v3(t_), bc(dis), v3(m0), ALU.subtract)
                fr = TD; TS(fr[:], t_[:], MAGIC2, MAGIC2, ALU.add, ALU.subtract)
                cl = TE; TT(cl[:], fr[:], t_[:], ALU.is_lt)
                k3 = TF; TT(k3[:], fr[:], cl[:], ALU.add)
                # probe pair in the reference's arithmetic:
                # u(k) = (k*vu) + u0 rounded at u-magnitude; crossed iff
                # q*(u >= c+d) + d  (asc: u>=c; desc: not(u>=c+1))
                cd = TA; TT(v3(cd), bc(descs), v3(cells8), ALU.add)
                pb = TB; TS(pb[:], k3[:], f(1.0), None, ALU.subtract)
                TT(v3(pb), v3(pb), bc(vus), ALU.mult)
                TT(v3(pb), v3(pb), bc(u0s), ALU.add)
                TT(pb[:], pb[:], cd[:], ALU.is_ge)
                TT(v3(pb), v3(pb), bc(qs), ALU.mult)
                TT(v3(pb), v3(pb), bc(descs), ALU.add)
                TT(k3[:], k3[:], pb[:], ALU.subtract)
                p2 = TC; TT(v3(p2), v3(k3), bc(vus), ALU.mult)
                TT(v3(p2), v3(p2), bc(u0s), ALU.add)
                TT(p2[:], p2[:], cd[:], ALU.is_ge)
                TT(v3(p2), v3(p2), bc(qs), ALU.mult)
                TT(v3(p2), v3(p2), bc(descs), ALU.add)
                TS(p2[:], p2[:], f(-1.0), f(1.0), ALU.mult, ALU.add)
                TT(k3[:], k3[:], p2[:], ALU.add)
                kf = TE; TS(kf[:], k3[:], f(0.0), f(200.0), ALU.max, ALU.min)
                vk = TG; TT(v3(vk), v3(k3), bc(khis), ALU.is_le)
                fu = TH; TT(v3(fu), v3(kf), bc(vus), ALU.mult)
                TT(v3(fu), v3(fu), bc(u0s), ALU.add)
                TT(fu[:], fu[:], cells8[:], ALU.subtract)
                w = TD; TT(v3(w), v3(kf), bc(vws), ALU.mult)
                TT(v3(w), v3(w), bc(w0s), ALU.add)
                if debug_dump and s_i == 0:
                    nc.sync.dma_start(dbg["FU"][:], fu[:])
                    nc.sync.dma_start(dbg["WW"][:], w[:])
                    nc.sync.dma_start(dbg["KF"][:], kf[:])
                va = TB; TS(va[:], fu[:], f(0.0), None, ALU.is_ge)
                vb = TC; TS(vb[:], fu[:], f(1.0), None, ALU.is_lt)
                TT(va[:], va[:], vb[:], ALU.mult)
                TT(va[:], va[:], vk[:], ALU.mult)
                if debug_dump and s_i == 0:
                    nc.sync.dma_start(dbg["VA"][:], va[:])
                # wcl = clamp(w, 0, 99) on ACT
                r0 = TE; nc.scalar.activation(r0[:], w[:], ACTF.Relu)
                r1 = TF; nc.scalar.activation(r1[:], r0[:], ACTF.Relu,
                                              bias=c99[:], scale=f(-1.0))
                wcl = TG; nc.scalar.activation(wcl[:], r1[:], ACTF.Identity,
                                               bias=c99[:], scale=f(-1.0))
                fr2 = TC; TS(fr2[:], wcl[:], MAGIC, MAGIC, ALU.add,
                             ALU.subtract)
                g2 = TE; TT(g2[:], fr2[:], wcl[:], ALU.is_gt)
                wf = TD; TT(wf[:], fr2[:], g2[:], ALU.subtract)
                cw = TI; TT(v3(cw), v3(va), bc(cs), ALU.mult,
                            eng=nc.gpsimd)
                fw = TB; TT(fw[:], wcl[:], wf[:], ALU.subtract)
                erel = TA; TT(erel[:], wf[:], eb8[:], ALU.add)
                fw1 = TE; nc.scalar.activation(fw1[:], fw[:], ACTF.Identity,
                                               bias=f(1.0), scale=f(-1.0))
                fu1 = TF; nc.scalar.activation(fu1[:], fu[:], ACTF.Identity,
                                               bias=f(1.0), scale=f(-1.0))
                a0 = TG; TT(a0[:], fw1[:], cw[:], ALU.mult)
                a1 = TC; TT(a1[:], fw[:], cw[:], ALU.mult)
                Wt = [P2.tile([NPASS, 800], F16, tag=f"W{j}",
                              name=f"W{j}", bufs=1) for j in range(4)]
                TT(Wt[0][:], a0[:], fu1[:], ALU.mult)
                TT(Wt[1][:], a1[:], fu1[:], ALU.mult)
                TT(Wt[2][:], a0[:], fu[:], ALU.mult)
                TT(Wt[3][:], a1[:], fu[:], ALU.mult)

                # ---- index wrap via PE double-transpose ----
                # T1: per g, erel g-block [128 pass, 100 c] -> [100 c, 128]
                # t1s columns ordered (p1, g, j): each p1-block of 128
                # columns is contiguous, so T2 is a plain [100,128] transpose.
                t1s = P2.tile([NCELL, 1024], F32, tag="t1s", name="t1s",
                              bufs=1)
                t1d = t1s[:].rearrange("p (q x) -> p q x", q=8)
                for _dup in range(2 if "wrap" in DUP else 1):
                    for g in range(8):
                        t1p = PS.tile([NCELL, 128], F32, tag="t1p",
                                      name="t1p", bufs=1)
                        nc.tensor.transpose(
                            t1p[:], erel[:, g * 100:g * 100 + 100], idn[:])
                        nc.scalar.activation(
                            t1d[:, :, g * 16:g * 16 + 16],
                            t1p[:].rearrange("p (q j) -> p q j", q=8),
                            ACTF.Copy)
                    for p1 in range(8):
                        wps = PS.tile([NPASS, NCELL], F32, tag="wps",
                                      name="wps", bufs=1)
                        nc.tensor.transpose(
                            wps[:], t1s[:, p1 * 128:p1 * 128 + 128],
                            idn[0:NCELL, 0:NCELL])
                        nc.vector.tensor_copy(idxw[:, p1::8], wps[:])

                # ---- weight transposes into one PSUM tile ----
                wtps = PS.tile([NCELL, 4096], F16, tag="wtps", name="wtps")
                for j in range(4):
                    for g in range(8):
                        col = (j * 8 + g) * 128
                        nc.tensor.transpose(
                            wtps[:, col:col + 128],
                            Wt[j][:, g * 100:g * 100 + 100], idnb[:])

                # absorb WAR deps (prior hop DMAs reading gout) + producers
                nc.gpsimd.tensor_scalar(
                    gout[:, 0:2].bitcast(F32), scratch[:, 0:1], f(0.0), None,
                    ALU.mult)
                nc.gpsimd.tensor_scalar(
                    scratch[:, 1:2], table_t[:, 0:2].bitcast(F32), f(0.0),
                    None, ALU.mult)
                nc.gpsimd.tensor_scalar(
                    scratch[:, 2:3], idxw[:, 0:2].bitcast(F32), f(0.0), None,
                    ALU.mult)
                nc.gpsimd.tensor_scalar(
                    scratch[:, 3:4], idxw[:, 798:800].bitcast(F32), f(0.0),
                    None, ALU.mult)
                for _dup in range(2 if "gather" in DUP else 1):
                    for c in range(NCHUNK):
                        i0 = c * CHUNK
                        i1 = min(NIDX, i0 + CHUNK)
                        basew = (i0 // 128) * 100
                        nc.gpsimd.indirect_copy(
                            gout[:, i0:i1],
                            table_t[:, basew:basew + WINDOW],
                            idxw[:, i0 // 16:i1 // 16],
                            i_know_ap_gather_is_preferred=True,
                        )
                nc.gpsimd.tensor_scalar(
                    scratch[:, 0:1], scratch[:, 0:1], f(1.0), None, ALU.mult)

                # ---- hop: gather rows -> HT0/HT1 (one tile per mask) ----
                ht0 = P2.tile([NCELL, 4096], F16, tag="ht0", name="ht0",
                              bufs=1)
                ht1 = P2.tile([NCELL, 4096], F16, tag="ht1", name="ht1",
                              bufs=1)
                for q8 in range(8):
                    htm = ht0 if q8 < 4 else ht1
                    for g in range(8):
                        col = ((q8 % 4) * 8 + g) * 128
                        eng = nc.sync if (q8 * 8 + g) % 2 == 0 else nc.scalar
                        eng.dma_start(
                            htm[:, col:col + 128],
                            gout[16 * g + q8:16 * g + q8 + 1, :]
                            .rearrange("o (n p) -> o n p", n=NCELL),
                        )
                wtsb = P2.tile([NCELL, 4096], F16, tag="wtsb",
                               name="wtsb", bufs=1)
                nc.scalar.activation(wtsb[:], wtps[:], ACTF.Copy)
                if debug_dump and s_i == 0:
                    nc.sync.dma_start(dbg["W0"][:], Wt[0][:])
                    nc.sync.dma_start(dbg["HT"][:, 0:4096], ht0[:])
                    nc.sync.dma_start(dbg["HT"][:, 4096:8192], ht1[:])
                    nc.sync.dma_start(dbg["WT"][:], wtsb[:])
                acc_dump = P2.tile([NCELL, 128], F16, tag="acc_dump",
                                   name="acc_dump", bufs=1)
                for h in range(2):
                    prod0 = P2.tile([NCELL, 2048], F32, tag="prod0",
                                    name="prod0", bufs=1)
                    TT(prod0[:], wtsb[:, h * 2048:h * 2048 + 2048],
                       ht0[:, h * 2048:h * 2048 + 2048], ALU.mult)
                    nc.vector.tensor_reduce(
                        collect[:, s_i * 64 + h * 16:s_i * 64 + h * 16 + 16],
                        prod0[:].rearrange("p (qg x) -> p qg x", x=128),
                        AXF, ALU.add)
                    prod1 = P2.tile([NCELL, 2048], F32, tag="prod1",
                                    name="prod1", bufs=1)
                    TT(prod1[:], wtsb[:, h * 2048:h * 2048 + 2048],
                       ht1[:, h * 2048:h * 2048 + 2048],
                       ALU.mult, eng=nc.gpsimd)
                    for qg in range(16):
                        col = s_i * 64 + 32 + h * 16 + qg
                        nc.scalar.activation(
                            acc_dump[:], prod1[:, qg * 128:qg * 128 + 128],
                            ACTF.Identity,
                            accum_out=collect[:, col:col + 1])
                # per-set slice of the final reduction on idle PE
                tps_s = PS.tile([1, 512], F32, tag="tps", name="tps")
                nc.tensor.matmul(tps_s[:, 0:64], ones[0:NCELL, :],
                                 collect[:, s_i * 64:s_i * 64 + 64])
                nc.vector.tensor_copy(tsum[:, s_i * 64:s_i * 64 + 64],
                                      tps_s[:, 0:64])

            if debug_dump:
                nc.sync.dma_start(dbg["CO"][:], collect[:])
            # ---------- final reduction (per-set matmuls emitted above) ----
            totN = nset * 64
            # cols = (set s, mask m, tap j, g); sum over j (4 taps)
            tv = tsum[:].rearrange("o (sm jg) -> o sm jg", jg=32)
            s2 = P.tile([1, nset * 16], F32, tag="s2", name="s2")
            s2v = s2[:].rearrange("o (sm g) -> o sm g", g=8)
            TT(s2v, tv[:, :, 0:8], tv[:, :, 8:16], ALU.add)
            TT(s2v, s2v, tv[:, :, 16:24], ALU.add)
            TT(s2v, s2v, tv[:, :, 24:32], ALU.add)
            sa_ = P.tile([1, nset * 16], F32, tag="sa_", name="sa_")
            nc.scalar.activation(sa_[:], s2[:], ACTF.Abs)
            # cols = (s, m, g): sum the two masks
            sv = sa_[:].rearrange("o (s mg) -> o s mg", mg=16)
            sm = P.tile([1, nset * 8], F32, tag="sm", name="sm")
            smv = sm[:].rearrange("o (s g) -> o s g", g=8)
            TT(smv, sv[:, :, 0:8], sv[:, :, 8:16], ALU.add)
            # cols = (ax, b): sum the two axes
            ia = P.tile([1, b_core], F32, tag="ia", name="ia")
            TT(ia[:], sm[:, 0:b_core], sm[:, b_core:2 * b_core], ALU.add)
            TS(ia[:], ia[:], f(0.25), None, ALU.mult)
            # areas [b x 1] -> [1 x b] via PE transpose
            area_r = []
            for nmtag, src in (("par", pred_a), ("gar", gt_a)):
                ptr = PS.tile([1, b_core], F32, tag="areap",
                              name=nmtag + "p")
                nc.tensor.transpose(ptr[:], src[:], idn[0:b_core, 0:b_core])
                r = P.tile([1, b_core], F32, tag=nmtag, name=nmtag)
                nc.vector.tensor_copy(r[:], ptr[:])
                area_r.append(r)
            un = P.tile([1, b_core], F32, tag="un", name="un")
            TT(un[:], area_r[0][:], area_r[1][:], ALU.add)
            TT(un[:], un[:], ia[:], ALU.subtract)
            rc = P.tile([1, b_core], F32, tag="rc", name="rc")
            nc.vector.reciprocal(rc[:], un[:])
            iou = P.tile([1, b_core], F32, tag="iou", name="iou")
            TT(iou[:], ia[:], rc[:], ALU.mult)
            nc.sync.dma_start(OUT[:], iou[:])
    _split_excess_waits(nc)
    return nc


def _pack_masks(m):
    """m: [b, 4, 100, 100] f32 -> bf16 tables [b, 2, 2, TAB_SRC].

    ax=0 holds transposed masks 0,1 (x-major flat); ax=1 masks 2,3
    (y-major flat)."""
    b = m.shape[0]
    out = np.zeros((b, 2, 2, TAB_SRC), np.float16)
    flat_x = np.transpose(m[:, 0:2], (0, 1, 3, 2)).reshape(b, 2, -1)
    flat_y = m[:, 2:4].reshape(b, 2, -1)
    out[:, 0, :, :10000] = flat_x.astype(np.float16)
    out[:, 1, :, :10000] = flat_y.astype(np.float16)
    return out


def _host_prep(poly, gt, gt_mask, b0, b_core):
    """One core's input map: relayout + constants (same level as v1)."""
    f32 = np.float32
    p = poly[b0:b0 + b_core].astype(f32)
    g = gt[b0:b0 + b_core].astype(f32)
    m = gt_mask[b0:b0 + b_core].astype(f32)
    pn = np.roll(p, -1, axis=1)
    gn = np.roll(g, -1, axis=1)

    def unit(a0, a1):
        return (a1 - a0 + f32(1e-6)).astype(f32)

    dxf = unit(p[:, :, 0], pn[:, :, 0]); dyf = unit(p[:, :, 1], pn[:, :, 1])
    nf = np.sqrt(dxf * dxf + dyf * dyf).astype(f32)
    vxf = (dxf / nf).astype(f32); vyf = (dyf / nf).astype(f32)
    dxb = unit(pn[:, :, 0], p[:, :, 0]); dyb = unit(pn[:, :, 1], p[:, :, 1])
    nb = np.sqrt(dxb * dxb + dyb * dyb).astype(f32)
    vxb = (dxb / nb).astype(f32); vyb = (dyb / nb).astype(f32)
    VX = np.concatenate([vxf.T, vxb.T], 0).astype(f32).copy()
    VY = np.concatenate([vyf.T, vyb.T], 0).astype(f32).copy()
    X0 = np.concatenate([p[:, :, 0].T, pn[:, :, 0].T], 0).astype(f32).copy()
    Y0 = np.concatenate([p[:, :, 1].T, pn[:, :, 1].T], 0).astype(f32).copy()
    X1 = np.concatenate([pn[:, :, 0].T, p[:, :, 0].T], 0).astype(f32).copy()
    Y1 = np.concatenate([pn[:, :, 1].T, p[:, :, 1].T], 0).astype(f32).copy()
    PAs = [p[:, :, 0], p[:, :, 1], pn[:, :, 0], pn[:, :, 1]]
    GAs = [g[:, :, 0], g[:, :, 1], gn[:, :, 0], gn[:, :, 1]]
    n_ = np.arange(NCELL, dtype=f32)
    CELLS8 = np.tile(n_, 8)[None, :].repeat(NPASS, 0).copy()
    EB8 = np.tile(100.0 * (n_ % 8).astype(f32), 8)[None, :].repeat(NPASS, 0).copy()
    FLIP = np.concatenate([np.ones((64, 1), f32), -np.ones((64, 1), f32)], 0)
    ONES = np.ones((NPASS, 1), f32)
    IDN = np.eye(NPASS, dtype=f32)
    IDNB = np.eye(NPASS).astype(np.float16)
    ret = {"X0": X0, "Y0": Y0, "X1": X1, "Y1": Y1, "VX": VX, "VY": VY,
           "TBL": _pack_masks(m), "CELLS8": CELLS8, "EB8": EB8,
           "FLIP": FLIP, "ONES": ONES, "IDN": IDN, "IDNB": IDNB}
    for i in range(4):
        ret[f"PA{i}"] = np.ascontiguousarray(PAs[i].astype(f32))
        ret[f"GA{i}"] = np.ascontiguousarray(GAs[i].astype(f32))
    return ret


def kernel(poly, gt, gt_mask):
    from concourse.bass_utils import run_bass_kernel_spmd
    poly = np.asarray(poly); gt = np.asarray(gt); gt_mask = np.asarray(gt_mask)
    bs = poly.shape[0]
    b_core = bs // N_CORES
    key = ("mod", b_core)
    if key not in _cache:
        _cache[key] = build_module(b_core)
    nc = _cache[key]
    in_maps = [_host_prep(poly, gt, gt_mask, c * b_core, b_core)
               for c in range(N_CORES)]
    res = run_bass_kernel_spmd(nc, in_maps, list(range(N_CORES)))
    out = np.concatenate([np.asarray(res.results[c]["IOU"]).reshape(-1)
                          for c in range(N_CORES)])
    return out.astype(np.float32)
